# revision 20
# baseline (speedup 1.0000x reference)
"""Trainium2 Bass kernel for nn_CrossHeadDeltaQuantizer.

Sharding: data-parallel over batch (B=8 -> 8 cores, core c owns batch c).

Per-core pipeline (quantization decisions need fp32-exact rotations: any
perturbation of the rotated values flips codebook decisions near boundaries,
and every anchor flip cascades into 7 delta-head rows, so the forward
rotations, their input transposes, and the ar path stay fp32; only
post-decision work is cheap):

  anchor (per 512-row block of head 0):
    ssq -> an (ACT sqrt + 1 Newton) -> inv -> xn = x*inv        [s,d]
    PE-T(xn) fp32 -> fwd mm fp32 (R_a^T stationary, 512-moving) [d',s]
    staircase quantize -> res -> |res|, sign(res)
    alpha_bc = (ones/D f32r) mm |res|  (partition reduce + broadcast in one)
    yhT = q0 + alpha_bc*sgn -> 4x direct [s,d] mm (yhT stationary)
    ar = wP * an   (resident; head-0 store)
  delta (per head 1..7, per block):
    dl = hd - ar -> dsq row norms (ACT square+accum) -> dn -> dn*h_eff
    PE-T(dl) fp32 -> fwd mm fp32 -> zT;  sgnT = ((zT>0)-0.5) bf16
    4x direct [s,d] mm (sgnT stationary, 2*R_d bf16 moving) -> oP
    ob = oP*dnh + ar  -> bf16 HBM store (heads 1-7 stored bf16)

Codebook scalars are baked into instruction immediates at trace time.
Fast path assumes the delta codebook is symmetric (d_sym) and R_d orthogonal
(both guaranteed by the reference's Lloyd-Max/QR construction); otherwise a
NumPy fallback computes the exact reference on host.
"""

import numpy as np
import ml_dtypes

from concourse import bass, bacc, tile, mybir
from concourse.bass_utils import run_bass_kernel_spmd

dt = mybir.dt
Alu = mybir.AluOpType
Act = mybir.ActivationFunctionType

B, H, S, D = 8, 8, 4096, 128
EPS = 1e-8
P = 128
TPB = 4                      # s-tiles per block
NBLK = S // (P * TPB)        # 8
HD = H - 1                   # delta heads

_CACHE = {}


def _f32(x):
    return float(np.float32(x))


def host_prep(R_anchor, cb_anchor, R_delta, cb_delta):
    R_a = np.ascontiguousarray(np.asarray(R_anchor, np.float32))
    R_d = np.ascontiguousarray(np.asarray(R_delta, np.float32))
    cb = np.asarray(cb_anchor, np.float32)
    cd = np.asarray(cb_delta, np.float32)

    p = {}
    p["r_a_t"] = np.ascontiguousarray(R_a.T)          # fwd anchor stationary
    p["r_a"] = R_a                                    # bwd anchor stationary
    p["r_d_t"] = np.ascontiguousarray(R_d.T)          # fwd delta stationary
    p["r_d_bf"] = (R_d.astype(ml_dtypes.bfloat16).astype(np.float32) * 2.0).astype(ml_dtypes.bfloat16)  # 2*R_d, bwd moving (sign path uses +-0.5)
    p["ones_d"] = np.full((P, P), 1.0 / D, dtype=np.float32)
    p["ident_f"] = np.eye(P, dtype=np.float32)

    order = np.argsort(cb, kind="stable")
    cs = cb[order]
    ts_, ge_, dl_ = [], [], []
    for i in range(len(cs) - 1):
        ts_.append(_f32((np.float32(cs[i]) + np.float32(cs[i + 1])) / np.float32(2)))
        ge_.append(bool(order[i + 1] < order[i]))
        dl_.append(_f32(np.float32(cs[i + 1]) - np.float32(cs[i])))
    p["a_ts"], p["a_ge"], p["a_dl"] = ts_, ge_, dl_
    p["a_c0"] = _f32(cs[0])

    c0, c1 = np.float32(cd[0]), np.float32(cd[1])
    k1 = np.float32(2.0) * (c1 - c0)
    k2 = c1 * c1 - c0 * c0
    p["d_h_eff"] = _f32(((c1 - c0) / np.float32(2)) * np.float32(np.sign(k1) if k1 != 0 else 1.0))
    p["d_sym"] = bool(k2 == np.float32(0.0))
    I = np.eye(D, dtype=np.float32)
    p["rd_orth"] = bool(np.abs(R_d @ R_d.T - I).max() < 1e-5)
    return p


def build(p):
    nc = bacc.Bacc()
    kv = nc.declare_dram_parameter("kv", [H, S, D], dt.float32, isOutput=False)
    r_a_t_d = nc.declare_dram_parameter("r_a_t", [D, D], dt.float32, isOutput=False)
    r_a_d = nc.declare_dram_parameter("r_a", [D, D], dt.float32, isOutput=False)
    r_d_t_d = nc.declare_dram_parameter("r_d_t", [D, D], dt.float32, isOutput=False)
    r_d_bf_d = nc.declare_dram_parameter("r_d_bf", [D, D], dt.bfloat16, isOutput=False)
    ones_d_d = nc.declare_dram_parameter("ones_d", [P, P], dt.float32r, isOutput=False)
    id_f_d = nc.declare_dram_parameter("ident_f", [P, P], dt.float32, isOutput=False)
    out0 = nc.declare_dram_parameter("out0", [S, D], dt.float32, isOutput=True)
    outd = nc.declare_dram_parameter("outd", [HD, S, D], dt.bfloat16, isOutput=True)

    def blk0(b):
        return kv[0][b * P * TPB:(b + 1) * P * TPB].rearrange("(j p) d -> p j d", p=P)

    def blkh(h, b):
        return kv[h][b * P * TPB:(b + 1) * P * TPB].rearrange("(j p) d -> p j d", p=P)

    def blk_out0(b):
        return out0[b * P * TPB:(b + 1) * P * TPB].rearrange("(j p) d -> p j d", p=P)

    def blk_outd(h, b):
        return outd[h - 1][b * P * TPB:(b + 1) * P * TPB].rearrange(
            "(j p) d -> p j d", p=P)

    with tile.TileContext(nc) as tc:
        with tc.tile_pool(name="consts", bufs=1) as cpool, \
             tc.tile_pool(name="resid", bufs=1) as rpool, \
             tc.tile_pool(name="stat", bufs=2) as spool, \
             tc.tile_pool(name="io", bufs=3) as iopool, \
             tc.tile_pool(name="wk", bufs=2) as wpool, \
             tc.tile_pool(name="junk", bufs=2) as jpool, \
             tc.tile_pool(name="psT", bufs=2, space="PSUM") as psT, \
             tc.tile_pool(name="psM", bufs=3, space="PSUM") as psM, \
             tc.tile_pool(name="psO", bufs=2, space="PSUM") as psO:

            # ---- constants ----
            r_a_t = cpool.tile([D, D], dt.float32, tag="c_rat")
            r_a = cpool.tile([D, D], dt.float32, tag="c_ra")
            r_d_t = cpool.tile([D, D], dt.float32, tag="c_rdt")
            r_d_bf = cpool.tile([D, D], dt.bfloat16, tag="c_rdbf")
            ones_t = cpool.tile([P, P], dt.float32r, tag="c_ones")
            id_f = cpool.tile([P, P], dt.float32, tag="c_idf")
            for t_, d_ in ((r_a_t, r_a_t_d), (r_a, r_a_d), (r_d_t, r_d_t_d),
                           (r_d_bf, r_d_bf_d), (ones_t, ones_d_d), (id_f, id_f_d)):
                nc.sync.dma_start(out=t_, in_=d_[:])
            halfc = cpool.tile([P, TPB, P], dt.float32, tag="c_half")
            nc.vector.memset(halfc, 0.5)

            # resident anchor reconstruction (f32r so PE can re-read it fast)
            ar = [rpool.tile([P, TPB, P], dt.float32, tag=f"ar{b}", name=f"ar{b}")
                  for b in range(NBLK)]
            an_all = rpool.tile([P, NBLK, TPB], dt.float32, tag="an_all")

            def sqrt1(pool, q, n, tagp):
                """sqrt(max(q,floor)) with one Newton step; returns (s1, parts)
                s1 = 0.5*(s0 + q/s0)."""
                t0 = pool.tile([P, n], dt.float32, tag=f"{tagp}_t0")
                s0 = pool.tile([P, n], dt.float32, tag=f"{tagp}_s0")
                r0 = pool.tile([P, n], dt.float32, tag=f"{tagp}_r0")
                nc.vector.tensor_scalar(out=t0, in0=q, scalar1=1e-35, scalar2=None,
                                        op0=Alu.max)
                nc.scalar.activation(out=s0, in_=t0, func=Act.Sqrt)
                nc.vector.reciprocal(out=r0, in_=s0)
                nc.vector.tensor_tensor(out=r0, in0=t0, in1=r0, op=Alu.mult)
                nc.vector.tensor_tensor(out=t0, in0=s0, in1=r0, op=Alu.add)
                return t0    # = 2*sqrt(q); fold the 0.5 into the next scale

            # ================= ANCHOR =================
            for b in range(NBLK):
                xa = iopool.tile([P, TPB, P], dt.float32, tag="xa")
                nc.sync.dma_start(out=xa, in_=blk0(b))
                ssq = spool.tile([P, TPB], dt.float32, tag="ssq")
                junk = jpool.tile([P, TPB, P], dt.float32, tag="junkA")
                for j in range(TPB):
                    nc.scalar.activation(out=junk[:, j], in_=xa[:, j],
                                         func=Act.Square,
                                         accum_out=ssq[:, j:j + 1])
                an2 = sqrt1(spool, ssq, TPB, "an")     # = 2*an
                # an_all slice = an = 0.5*an2
                nc.vector.tensor_scalar(out=an_all[:, b], in0=an2, scalar1=0.5,
                                        scalar2=None, op0=Alu.mult)
                inv = spool.tile([P, TPB], dt.float32, tag="inv")
                nc.vector.tensor_scalar(out=inv, in0=an_all[:, b],
                                        scalar1=_f32(EPS), scalar2=None,
                                        op0=Alu.add)
                nc.vector.reciprocal(out=inv, in_=inv)

                xn = wpool.tile([P, TPB, P], dt.float32, tag="xn")
                inv_bc = inv[:, :, None].broadcast_to([P, TPB, P])
                nc.vector.tensor_tensor(out=xn, in0=xa, in1=inv_bc, op=Alu.mult)

                pT = psT.tile([P, TPB, P], dt.float32, tag="pT", name="pTa")
                for j in range(TPB):
                    nc.tensor.transpose(pT[:, j], xn[:, j], id_f)
                xnT = wpool.tile([P, TPB, P], dt.float32, tag="xnT")
                nc.scalar.activation(out=xnT, in_=pT, func=Act.Copy)

                yP = psM.tile([P, TPB * P], dt.float32, tag="mm", name="yP")
                nc.tensor.matmul(yP, lhsT=r_a_t,
                                 rhs=xnT.rearrange("p j q -> p (j q)"),
                                 start=True, stop=True)

                # staircase quantize in [d', s] domain
                a1 = wpool.tile([P, TPB * P], dt.float32, tag="a1")
                a2 = wpool.tile([P, TPB * P], dt.float32, tag="a2")
                a3 = wpool.tile([P, TPB * P], dt.float32, tag="a3")
                for ai, (tt, ge, dl) in zip(
                        (a1, a2, a3),
                        zip(p["a_ts"], p["a_ge"], p["a_dl"])):
                    nc.vector.tensor_scalar(out=ai, in0=yP, scalar1=tt,
                                            scalar2=dl,
                                            op0=(Alu.is_ge if ge else Alu.is_gt),
                                            op1=Alu.mult)
                q0 = wpool.tile([P, TPB * P], dt.float32, tag="q0")
                nc.vector.scalar_tensor_tensor(out=q0, in0=a1, scalar=p["a_c0"],
                                               in1=a2, op0=Alu.add, op1=Alu.add)
                nc.gpsimd.tensor_tensor(out=q0, in0=q0, in1=a3, op=Alu.add)
                res = wpool.tile([P, TPB * P], dt.float32, tag="res")
                nc.vector.tensor_tensor(out=res, in0=yP, in1=q0, op=Alu.subtract)
                ares = wpool.tile([P, TPB * P], dt.float32r, tag="ares")
                nc.scalar.activation(out=ares, in_=res, func=Act.Abs)
                sgn = wpool.tile([P, TPB * P], dt.float32, tag="sgnA")
                nc.scalar.activation(out=sgn, in_=res, func=Act.Sign)

                aP = psM.tile([P, TPB * P], dt.float32, tag="mm", name="aP")
                nc.tensor.matmul(aP, lhsT=ones_t, rhs=ares, start=True, stop=True)

                v = wpool.tile([P, TPB * P], dt.float32, tag="vA")
                nc.vector.tensor_tensor(out=v, in0=aP, in1=sgn, op=Alu.mult)
                yhT = wpool.tile([P, TPB * P], dt.float32, tag="yhT")
                nc.gpsimd.tensor_tensor(out=yhT, in0=q0, in1=v, op=Alu.add)

                yhT3 = yhT.rearrange("p (j q) -> p j q", j=TPB)
                wP4 = psO.tile([P, TPB, P], dt.float32, tag="oP", name="wP4")
                for j in range(TPB):
                    nc.tensor.matmul(wP4[:, j], lhsT=yhT3[:, j], rhs=r_a,
                                     start=True, stop=True)
                an_bc = an_all[:, b][:, :, None].broadcast_to([P, TPB, P])
                nc.vector.tensor_tensor(out=ar[b], in0=wP4, in1=an_bc, op=Alu.mult)
                nc.sync.dma_start(out=blk_out0(b), in_=ar[b])

            # ================= DELTA HEADS =================
            for b in range(NBLK):
                # phase 1: all heads' delta + row norms
                deltas = []
                dsq = spool.tile([P, HD, TPB], dt.float32, tag="dsq")
                for h in range(1, H):
                    hd = iopool.tile([P, TPB, P], dt.float32, tag="hd",
                                     name=f"hd{h}")
                    nc.sync.dma_start(out=hd, in_=blkh(h, b))
                    dl = wpool.tile([P, TPB, P], dt.float32, tag=f"dl{h}",
                                    name=f"dl{h}")
                    nc.vector.tensor_tensor(out=dl, in0=hd, in1=ar[b],
                                            op=Alu.subtract)
                    junk = jpool.tile([P, TPB, P], dt.float32, tag="junkD")
                    for j in range(TPB):
                        nc.scalar.activation(out=junk[:, j], in_=dl[:, j],
                                             func=Act.Square,
                                             accum_out=dsq[:, h - 1, j:j + 1])
                    deltas.append(dl)
                # dn chain on [P, HD*TPB]: dnh = h_eff * sqrt(dsq)
                dsq_f = dsq.rearrange("p h j -> p (h j)")
                dn2 = sqrt1(spool, dsq_f, HD * TPB, "dn")   # = 2*dn
                dnh = spool.tile([P, HD, TPB], dt.float32, tag="dnh")
                nc.vector.tensor_scalar(
                    out=dnh.rearrange("p h j -> p (h j)"), in0=dn2,
                    scalar1=_f32(p["d_h_eff"] * 0.5), scalar2=None, op0=Alu.mult)

                # phase 2: software-pipelined — T(h+1) and fwd(h) are
                # issued ahead of bwd(h-1) so the PE never waits on the ACT
                # evacuation or the DVE sign op.
                def emit_T(h):
                    dl = deltas[h - 1]
                    pT = psT.tile([P, TPB, P], dt.float32, tag="pT",
                                  name=f"pTd{h}")
                    for j in range(TPB):
                        nc.tensor.transpose(pT[:, j], dl[:, j], id_f)
                    return pT

                def emit_fwd(h, pT):
                    dT = wpool.tile([P, TPB, P], dt.float32, tag="dT",
                                    name=f"dT{h}")
                    nc.scalar.activation(out=dT, in_=pT, func=Act.Copy)
                    zP = psM.tile([P, TPB * P], dt.float32, tag="mm",
                                  name=f"zP{h}")
                    nc.tensor.matmul(zP, lhsT=r_d_t,
                                     rhs=dT.rearrange("p j q -> p (j q)"),
                                     start=True, stop=True)
                    sgnT = wpool.tile([P, TPB, P], dt.bfloat16, tag="sgnT",
                                      name=f"sgnT{h}", bufs=3)
                    nc.vector.scalar_tensor_tensor(
                        out=sgnT, in0=zP.rearrange("p (j q) -> p j q", j=TPB),
                        scalar=0.0, in1=halfc, op0=Alu.is_gt, op1=Alu.subtract)
                    return sgnT

                def emit_bwd(h, sgnT):
                    oP = psO.tile([P, TPB, P], dt.float32, tag="oP",
                                  name=f"oP{h}")
                    for j in range(TPB):
                        nc.tensor.matmul(oP[:, j], lhsT=sgnT[:, j], rhs=r_d_bf,
                                         start=True, stop=True)
                    tD = wpool.tile([P, TPB, P], dt.float32, tag="tD")
                    dnh_bc = dnh[:, h - 1][:, :, None].broadcast_to([P, TPB, P])
                    nc.vector.tensor_tensor(out=tD, in0=oP, in1=dnh_bc,
                                            op=Alu.mult)
                    ob = wpool.tile([P, TPB, P], dt.bfloat16, tag="ob")
                    nc.gpsimd.tensor_tensor(out=ob, in0=tD, in1=ar[b],
                                            op=Alu.add)
                    nc.sync.dma_start(out=blk_outd(h, b), in_=ob)

                pT_cur = emit_T(1)
                sgnTs = {}
                for h in range(1, H):
                    sgnTs[h] = emit_fwd(h, pT_cur)
                    if h < H - 1:
                        pT_cur = emit_T(h + 1)
                    if h >= 2:
                        emit_bwd(h - 1, sgnTs.pop(h - 1))
                emit_bwd(H - 1, sgnTs.pop(H - 1))
    nc.finalize()
    return nc


def _numpy_fallback(kv_states, R_anchor, cb_anchor, R_delta, cb_delta):
    kv = np.asarray(kv_states, np.float32)
    b, h, s, d = kv.shape
    R_a = np.asarray(R_anchor, np.float32)
    R_d = np.asarray(R_delta, np.float32)
    cb = np.asarray(cb_anchor, np.float32)
    cd = np.asarray(cb_delta, np.float32)

    anchor = kv[:, 0].reshape(-1, d)
    an = np.linalg.norm(anchor, axis=-1, keepdims=True)
    y = (anchor / (an + EPS)) @ R_a.T
    idx = np.argmin((y[..., None] - cb) ** 2, axis=-1)
    q0 = cb[idx]
    res = y - q0
    alpha = np.mean(np.abs(res), axis=-1, keepdims=True)
    y_hat = q0 + alpha * np.sign(res)
    ar = ((y_hat @ R_a) * an).reshape(b, s, d)

    delta = kv[:, 1:] - ar[:, None]
    dn = np.linalg.norm(delta, axis=-1, keepdims=True)
    yd = (delta / (dn + EPS)) @ R_d.T
    didx = np.argmin((yd[..., None] - cd) ** 2, axis=-1)
    drecon = cd[didx] @ R_d
    out = np.empty_like(kv)
    out[:, 0] = ar
    out[:, 1:] = ar[:, None] + drecon * dn
    return out


def kernel(**inputs):
    kv_states = np.ascontiguousarray(np.asarray(inputs["kv_states"], np.float32))
    p = host_prep(inputs["R_anchor"], inputs["cb_anchor"],
                  inputs["R_delta"], inputs["cb_delta"])
    if not (p["d_sym"] and p["rd_orth"] and len(p["a_ts"]) == 3):
        return _numpy_fallback(kv_states, inputs["R_anchor"],
                               inputs["cb_anchor"], inputs["R_delta"],
                               inputs["cb_delta"])

    key = (tuple(p["a_ts"]), tuple(p["a_ge"]), tuple(p["a_dl"]), p["a_c0"],
           p["d_h_eff"])
    if key not in _CACHE:
        _CACHE[key] = build(p)
    nc = _CACHE[key]

    shared = {k: p[k] for k in ("r_a_t", "r_a", "r_d_t", "r_d_bf",
                                "ones_d", "ident_f")}
    in_maps = [dict(shared, kv=kv_states[c]) for c in range(B)]
    res = run_bass_kernel_spmd(nc, in_maps, core_ids=list(range(B)))
    out = np.empty((B, H, S, D), dtype=np.float32)
    for c in range(B):
        out[c, 0] = res.results[c]["out0"]
        out[c, 1:] = np.asarray(res.results[c]["outd"]).astype(np.float32)
    return out


if __name__ == "__main__":
    rng = np.random.default_rng(0)
    q, _ = np.linalg.qr(rng.standard_normal((D, D)))
    q2, _ = np.linalg.qr(rng.standard_normal((D, D)))
    fake = {
        "kv_states": rng.standard_normal((B, H, S, D)).astype(np.float32),
        "R_anchor": q.astype(np.float32),
        "cb_anchor": np.array([-0.1017, -0.0282, 0.0282, 0.1017], np.float32),
        "R_delta": q2.astype(np.float32),
        "cb_delta": np.array([-0.0596, 0.0596], np.float32),
    }
    o = kernel(**fake)
    print("ran", o.shape, o.dtype)


# revision 21
# speedup vs baseline: 1.0034x; 1.0034x over previous
"""Trainium2 Bass kernel for nn_CrossHeadDeltaQuantizer.

Sharding: data-parallel over batch (B=8 -> 8 cores, core c owns batch c).

Per-core pipeline (quantization decisions need fp32-exact rotations: any
perturbation of the rotated values flips codebook decisions near boundaries,
and every anchor flip cascades into 7 delta-head rows, so the forward
rotations, their input transposes, and the ar path stay fp32; only
post-decision work is cheap):

  anchor (per 512-row block of head 0):
    ssq -> an (ACT sqrt + 1 Newton) -> inv -> xn = x*inv        [s,d]
    PE-T(xn) fp32 -> fwd mm fp32 (R_a^T stationary, 512-moving) [d',s]
    staircase quantize -> res -> |res|, sign(res)
    alpha_bc = (ones/D f32r) mm |res|  (partition reduce + broadcast in one)
    yhT = q0 + alpha_bc*sgn -> 4x direct [s,d] mm (yhT stationary)
    ar = wP * an   (resident; head-0 store)
  delta (per head 1..7, per block):
    dl = hd - ar -> dsq row norms (ACT square+accum) -> dn -> dn*h_eff
    PE-T(dl) fp32 -> fwd mm fp32 -> zT;  sgnT = ((zT>0)-0.5) bf16
    4x direct [s,d] mm (sgnT stationary, 2*R_d bf16 moving) -> oP
    ob = oP*dnh + ar  -> bf16 HBM store (heads 1-7 stored bf16)

Codebook scalars are baked into instruction immediates at trace time.
Fast path assumes the delta codebook is symmetric (d_sym) and R_d orthogonal
(both guaranteed by the reference's Lloyd-Max/QR construction); otherwise a
NumPy fallback computes the exact reference on host.
"""

import numpy as np
import ml_dtypes

from concourse import bass, bacc, tile, mybir
from concourse.bass_utils import run_bass_kernel_spmd

dt = mybir.dt
Alu = mybir.AluOpType
Act = mybir.ActivationFunctionType

B, H, S, D = 8, 8, 4096, 128
EPS = 1e-8
P = 128
TPB = 4                      # s-tiles per block
NBLK = S // (P * TPB)        # 8
HD = H - 1                   # delta heads

_CACHE = {}


def _f32(x):
    return float(np.float32(x))


def host_prep(R_anchor, cb_anchor, R_delta, cb_delta):
    R_a = np.ascontiguousarray(np.asarray(R_anchor, np.float32))
    R_d = np.ascontiguousarray(np.asarray(R_delta, np.float32))
    cb = np.asarray(cb_anchor, np.float32)
    cd = np.asarray(cb_delta, np.float32)

    p = {}
    p["r_a_t"] = np.ascontiguousarray(R_a.T)          # fwd anchor stationary
    p["r_a"] = R_a                                    # bwd anchor stationary
    p["r_d_t"] = np.ascontiguousarray(R_d.T)          # fwd delta stationary
    p["r_d_bf"] = (R_d.astype(ml_dtypes.bfloat16).astype(np.float32) * 2.0).astype(ml_dtypes.bfloat16)  # 2*R_d, bwd moving (sign path uses +-0.5)
    p["ones_d"] = np.full((P, P), 1.0 / D, dtype=np.float32)
    p["ident_f"] = np.eye(P, dtype=np.float32)

    order = np.argsort(cb, kind="stable")
    cs = cb[order]
    ts_, ge_, dl_ = [], [], []
    for i in range(len(cs) - 1):
        ts_.append(_f32((np.float32(cs[i]) + np.float32(cs[i + 1])) / np.float32(2)))
        ge_.append(bool(order[i + 1] < order[i]))
        dl_.append(_f32(np.float32(cs[i + 1]) - np.float32(cs[i])))
    p["a_ts"], p["a_ge"], p["a_dl"] = ts_, ge_, dl_
    p["a_c0"] = _f32(cs[0])

    c0, c1 = np.float32(cd[0]), np.float32(cd[1])
    k1 = np.float32(2.0) * (c1 - c0)
    k2 = c1 * c1 - c0 * c0
    p["d_h_eff"] = _f32(((c1 - c0) / np.float32(2)) * np.float32(np.sign(k1) if k1 != 0 else 1.0))
    p["d_sym"] = bool(k2 == np.float32(0.0))
    I = np.eye(D, dtype=np.float32)
    p["rd_orth"] = bool(np.abs(R_d @ R_d.T - I).max() < 1e-5)
    return p


def build(p):
    nc = bacc.Bacc()
    kv = nc.declare_dram_parameter("kv", [H, S, D], dt.float32, isOutput=False)
    r_a_t_d = nc.declare_dram_parameter("r_a_t", [D, D], dt.float32, isOutput=False)
    r_a_d = nc.declare_dram_parameter("r_a", [D, D], dt.float32, isOutput=False)
    r_d_t_d = nc.declare_dram_parameter("r_d_t", [D, D], dt.float32, isOutput=False)
    r_d_bf_d = nc.declare_dram_parameter("r_d_bf", [D, D], dt.bfloat16, isOutput=False)
    ones_d_d = nc.declare_dram_parameter("ones_d", [P, P], dt.float32r, isOutput=False)
    id_f_d = nc.declare_dram_parameter("ident_f", [P, P], dt.float32, isOutput=False)
    out0 = nc.declare_dram_parameter("out0", [S, D], dt.float32, isOutput=True)
    outd = nc.declare_dram_parameter("outd", [HD, S, D], dt.bfloat16, isOutput=True)

    def blk0(b):
        return kv[0][b * P * TPB:(b + 1) * P * TPB].rearrange("(j p) d -> p j d", p=P)

    def blkh(h, b):
        return kv[h][b * P * TPB:(b + 1) * P * TPB].rearrange("(j p) d -> p j d", p=P)

    def blk_out0(b):
        return out0[b * P * TPB:(b + 1) * P * TPB].rearrange("(j p) d -> p j d", p=P)

    def blk_outd(h, b):
        return outd[h - 1][b * P * TPB:(b + 1) * P * TPB].rearrange(
            "(j p) d -> p j d", p=P)

    with tile.TileContext(nc) as tc:
        with tc.tile_pool(name="consts", bufs=1) as cpool, \
             tc.tile_pool(name="resid", bufs=1) as rpool, \
             tc.tile_pool(name="stat", bufs=2) as spool, \
             tc.tile_pool(name="io", bufs=4) as iopool, \
             tc.tile_pool(name="wk", bufs=2) as wpool, \
             tc.tile_pool(name="junk", bufs=2) as jpool, \
             tc.tile_pool(name="psT", bufs=2, space="PSUM") as psT, \
             tc.tile_pool(name="psM", bufs=3, space="PSUM") as psM, \
             tc.tile_pool(name="psO", bufs=3, space="PSUM") as psO:

            # ---- constants ----
            r_a_t = cpool.tile([D, D], dt.float32, tag="c_rat")
            r_a = cpool.tile([D, D], dt.float32, tag="c_ra")
            r_d_t = cpool.tile([D, D], dt.float32, tag="c_rdt")
            r_d_bf = cpool.tile([D, D], dt.bfloat16, tag="c_rdbf")
            ones_t = cpool.tile([P, P], dt.float32r, tag="c_ones")
            id_f = cpool.tile([P, P], dt.float32, tag="c_idf")
            for t_, d_ in ((r_a_t, r_a_t_d), (r_a, r_a_d), (r_d_t, r_d_t_d),
                           (r_d_bf, r_d_bf_d), (ones_t, ones_d_d), (id_f, id_f_d)):
                nc.sync.dma_start(out=t_, in_=d_[:])
            halfc = cpool.tile([P, TPB, P], dt.float32, tag="c_half")
            nc.vector.memset(halfc, 0.5)

            # resident anchor reconstruction (f32r so PE can re-read it fast)
            ar = [rpool.tile([P, TPB, P], dt.float32, tag=f"ar{b}", name=f"ar{b}")
                  for b in range(NBLK)]
            an_all = rpool.tile([P, NBLK, TPB], dt.float32, tag="an_all")

            def sqrt1(pool, q, n, tagp):
                """sqrt(max(q,floor)) with one Newton step; returns (s1, parts)
                s1 = 0.5*(s0 + q/s0)."""
                t0 = pool.tile([P, n], dt.float32, tag=f"{tagp}_t0")
                s0 = pool.tile([P, n], dt.float32, tag=f"{tagp}_s0")
                r0 = pool.tile([P, n], dt.float32, tag=f"{tagp}_r0")
                nc.vector.tensor_scalar(out=t0, in0=q, scalar1=1e-35, scalar2=None,
                                        op0=Alu.max)
                nc.scalar.activation(out=s0, in_=t0, func=Act.Sqrt)
                nc.vector.reciprocal(out=r0, in_=s0)
                nc.vector.tensor_tensor(out=r0, in0=t0, in1=r0, op=Alu.mult)
                nc.vector.tensor_tensor(out=t0, in0=s0, in1=r0, op=Alu.add)
                return t0    # = 2*sqrt(q); fold the 0.5 into the next scale

            # ================= ANCHOR =================
            for b in range(NBLK):
                xa = iopool.tile([P, TPB, P], dt.float32, tag="xa")
                nc.sync.dma_start(out=xa, in_=blk0(b))
                ssq = spool.tile([P, TPB], dt.float32, tag="ssq")
                junk = jpool.tile([P, TPB, P], dt.float32, tag="junkA")
                for j in range(TPB):
                    nc.scalar.activation(out=junk[:, j], in_=xa[:, j],
                                         func=Act.Square,
                                         accum_out=ssq[:, j:j + 1])
                an2 = sqrt1(spool, ssq, TPB, "an")     # = 2*an
                # an_all slice = an = 0.5*an2
                nc.vector.tensor_scalar(out=an_all[:, b], in0=an2, scalar1=0.5,
                                        scalar2=None, op0=Alu.mult)
                inv = spool.tile([P, TPB], dt.float32, tag="inv")
                nc.vector.tensor_scalar(out=inv, in0=an_all[:, b],
                                        scalar1=_f32(EPS), scalar2=None,
                                        op0=Alu.add)
                nc.vector.reciprocal(out=inv, in_=inv)

                xn = wpool.tile([P, TPB, P], dt.float32, tag="xn")
                inv_bc = inv[:, :, None].broadcast_to([P, TPB, P])
                nc.vector.tensor_tensor(out=xn, in0=xa, in1=inv_bc, op=Alu.mult)

                pT = psT.tile([P, TPB, P], dt.float32, tag="pT", name="pTa")
                for j in range(TPB):
                    nc.tensor.transpose(pT[:, j], xn[:, j], id_f)
                xnT = wpool.tile([P, TPB, P], dt.float32, tag="xnT")
                nc.scalar.activation(out=xnT, in_=pT, func=Act.Copy)

                yP = psM.tile([P, TPB * P], dt.float32, tag="mm", name="yP")
                nc.tensor.matmul(yP, lhsT=r_a_t,
                                 rhs=xnT.rearrange("p j q -> p (j q)"),
                                 start=True, stop=True)

                # staircase quantize in [d', s] domain
                a1 = wpool.tile([P, TPB * P], dt.float32, tag="a1")
                a2 = wpool.tile([P, TPB * P], dt.float32, tag="a2")
                a3 = wpool.tile([P, TPB * P], dt.float32, tag="a3")
                for ai, (tt, ge, dl) in zip(
                        (a1, a2, a3),
                        zip(p["a_ts"], p["a_ge"], p["a_dl"])):
                    nc.vector.tensor_scalar(out=ai, in0=yP, scalar1=tt,
                                            scalar2=dl,
                                            op0=(Alu.is_ge if ge else Alu.is_gt),
                                            op1=Alu.mult)
                q0 = wpool.tile([P, TPB * P], dt.float32, tag="q0")
                nc.vector.scalar_tensor_tensor(out=q0, in0=a1, scalar=p["a_c0"],
                                               in1=a2, op0=Alu.add, op1=Alu.add)
                nc.gpsimd.tensor_tensor(out=q0, in0=q0, in1=a3, op=Alu.add)
                res = wpool.tile([P, TPB * P], dt.float32, tag="res")
                nc.vector.tensor_tensor(out=res, in0=yP, in1=q0, op=Alu.subtract)
                ares = wpool.tile([P, TPB * P], dt.float32r, tag="ares")
                nc.scalar.activation(out=ares, in_=res, func=Act.Abs)
                sgn = wpool.tile([P, TPB * P], dt.float32, tag="sgnA")
                nc.scalar.activation(out=sgn, in_=res, func=Act.Sign)

                aP = psM.tile([P, TPB * P], dt.float32, tag="mm", name="aP")
                nc.tensor.matmul(aP, lhsT=ones_t, rhs=ares, start=True, stop=True)

                v = wpool.tile([P, TPB * P], dt.float32, tag="vA")
                nc.vector.tensor_tensor(out=v, in0=aP, in1=sgn, op=Alu.mult)
                yhT = wpool.tile([P, TPB * P], dt.float32, tag="yhT")
                nc.gpsimd.tensor_tensor(out=yhT, in0=q0, in1=v, op=Alu.add)

                yhT3 = yhT.rearrange("p (j q) -> p j q", j=TPB)
                wP4 = psO.tile([P, TPB, P], dt.float32, tag="oP", name="wP4")
                for j in range(TPB):
                    nc.tensor.matmul(wP4[:, j], lhsT=yhT3[:, j], rhs=r_a,
                                     start=True, stop=True)
                an_bc = an_all[:, b][:, :, None].broadcast_to([P, TPB, P])
                nc.vector.tensor_tensor(out=ar[b], in0=wP4, in1=an_bc, op=Alu.mult)
                nc.sync.dma_start(out=blk_out0(b), in_=ar[b])

            # ================= DELTA HEADS =================
            for b in range(NBLK):
                # phase 1: all heads' delta + row norms
                deltas = []
                dsq = spool.tile([P, HD, TPB], dt.float32, tag="dsq")
                for h in range(1, H):
                    hd = iopool.tile([P, TPB, P], dt.float32, tag="hd",
                                     name=f"hd{h}")
                    nc.sync.dma_start(out=hd, in_=blkh(h, b))
                    dl = wpool.tile([P, TPB, P], dt.float32, tag=f"dl{h}",
                                    name=f"dl{h}")
                    nc.vector.tensor_tensor(out=dl, in0=hd, in1=ar[b],
                                            op=Alu.subtract)
                    junk = jpool.tile([P, TPB, P], dt.float32, tag="junkD")
                    for j in range(TPB):
                        nc.scalar.activation(out=junk[:, j], in_=dl[:, j],
                                             func=Act.Square,
                                             accum_out=dsq[:, h - 1, j:j + 1])
                    deltas.append(dl)
                # dn chain on [P, HD*TPB]: dnh = h_eff * sqrt(dsq)
                dsq_f = dsq.rearrange("p h j -> p (h j)")
                dn2 = sqrt1(spool, dsq_f, HD * TPB, "dn")   # = 2*dn
                dnh = spool.tile([P, HD, TPB], dt.float32, tag="dnh")
                nc.vector.tensor_scalar(
                    out=dnh.rearrange("p h j -> p (h j)"), in0=dn2,
                    scalar1=_f32(p["d_h_eff"] * 0.5), scalar2=None, op0=Alu.mult)

                # phase 2: software-pipelined — T(h+1) and fwd(h) are
                # issued ahead of bwd(h-1) so the PE never waits on the ACT
                # evacuation or the DVE sign op.
                def emit_T(h):
                    dl = deltas[h - 1]
                    pT = psT.tile([P, TPB, P], dt.float32, tag="pT",
                                  name=f"pTd{h}")
                    for j in range(TPB):
                        nc.tensor.transpose(pT[:, j], dl[:, j], id_f)
                    return pT

                def emit_fwd(h, pT):
                    dT = wpool.tile([P, TPB, P], dt.float32, tag="dT",
                                    name=f"dT{h}")
                    nc.scalar.activation(out=dT, in_=pT, func=Act.Copy)
                    zP = psM.tile([P, TPB * P], dt.float32, tag="mm",
                                  name=f"zP{h}")
                    nc.tensor.matmul(zP, lhsT=r_d_t,
                                     rhs=dT.rearrange("p j q -> p (j q)"),
                                     start=True, stop=True)
                    sgnT = wpool.tile([P, TPB, P], dt.bfloat16, tag="sgnT",
                                      name=f"sgnT{h}", bufs=3)
                    nc.vector.scalar_tensor_tensor(
                        out=sgnT, in0=zP.rearrange("p (j q) -> p j q", j=TPB),
                        scalar=0.0, in1=halfc, op0=Alu.is_gt, op1=Alu.subtract)
                    return sgnT

                def emit_bwd(h, sgnT):
                    oP = psO.tile([P, TPB, P], dt.float32, tag="oP",
                                  name=f"oP{h}")
                    for j in range(TPB):
                        nc.tensor.matmul(oP[:, j], lhsT=sgnT[:, j], rhs=r_d_bf,
                                         start=True, stop=True)
                    tD = wpool.tile([P, TPB, P], dt.float32, tag="tD")
                    dnh_bc = dnh[:, h - 1][:, :, None].broadcast_to([P, TPB, P])
                    nc.vector.tensor_tensor(out=tD, in0=oP, in1=dnh_bc,
                                            op=Alu.mult)
                    ob = wpool.tile([P, TPB, P], dt.bfloat16, tag="ob")
                    nc.gpsimd.tensor_tensor(out=ob, in0=tD, in1=ar[b],
                                            op=Alu.add)
                    nc.sync.dma_start(out=blk_outd(h, b), in_=ob)

                pT_cur = emit_T(1)
                sgnTs = {}
                for h in range(1, H):
                    sgnTs[h] = emit_fwd(h, pT_cur)
                    if h < H - 1:
                        pT_cur = emit_T(h + 1)
                    if h >= 2:
                        emit_bwd(h - 1, sgnTs.pop(h - 1))
                emit_bwd(H - 1, sgnTs.pop(H - 1))
    nc.finalize()
    return nc


def _numpy_fallback(kv_states, R_anchor, cb_anchor, R_delta, cb_delta):
    kv = np.asarray(kv_states, np.float32)
    b, h, s, d = kv.shape
    R_a = np.asarray(R_anchor, np.float32)
    R_d = np.asarray(R_delta, np.float32)
    cb = np.asarray(cb_anchor, np.float32)
    cd = np.asarray(cb_delta, np.float32)

    anchor = kv[:, 0].reshape(-1, d)
    an = np.linalg.norm(anchor, axis=-1, keepdims=True)
    y = (anchor / (an + EPS)) @ R_a.T
    idx = np.argmin((y[..., None] - cb) ** 2, axis=-1)
    q0 = cb[idx]
    res = y - q0
    alpha = np.mean(np.abs(res), axis=-1, keepdims=True)
    y_hat = q0 + alpha * np.sign(res)
    ar = ((y_hat @ R_a) * an).reshape(b, s, d)

    delta = kv[:, 1:] - ar[:, None]
    dn = np.linalg.norm(delta, axis=-1, keepdims=True)
    yd = (delta / (dn + EPS)) @ R_d.T
    didx = np.argmin((yd[..., None] - cd) ** 2, axis=-1)
    drecon = cd[didx] @ R_d
    out = np.empty_like(kv)
    out[:, 0] = ar
    out[:, 1:] = ar[:, None] + drecon * dn
    return out


def kernel(**inputs):
    kv_states = np.ascontiguousarray(np.asarray(inputs["kv_states"], np.float32))
    p = host_prep(inputs["R_anchor"], inputs["cb_anchor"],
                  inputs["R_delta"], inputs["cb_delta"])
    if not (p["d_sym"] and p["rd_orth"] and len(p["a_ts"]) == 3):
        return _numpy_fallback(kv_states, inputs["R_anchor"],
                               inputs["cb_anchor"], inputs["R_delta"],
                               inputs["cb_delta"])

    key = (tuple(p["a_ts"]), tuple(p["a_ge"]), tuple(p["a_dl"]), p["a_c0"],
           p["d_h_eff"])
    if key not in _CACHE:
        _CACHE[key] = build(p)
    nc = _CACHE[key]

    shared = {k: p[k] for k in ("r_a_t", "r_a", "r_d_t", "r_d_bf",
                                "ones_d", "ident_f")}
    in_maps = [dict(shared, kv=kv_states[c]) for c in range(B)]
    res = run_bass_kernel_spmd(nc, in_maps, core_ids=list(range(B)))
    out = np.empty((B, H, S, D), dtype=np.float32)
    for c in range(B):
        out[c, 0] = res.results[c]["out0"]
        out[c, 1:] = np.asarray(res.results[c]["outd"]).astype(np.float32)
    return out


if __name__ == "__main__":
    rng = np.random.default_rng(0)
    q, _ = np.linalg.qr(rng.standard_normal((D, D)))
    q2, _ = np.linalg.qr(rng.standard_normal((D, D)))
    fake = {
        "kv_states": rng.standard_normal((B, H, S, D)).astype(np.float32),
        "R_anchor": q.astype(np.float32),
        "cb_anchor": np.array([-0.1017, -0.0282, 0.0282, 0.1017], np.float32),
        "R_delta": q2.astype(np.float32),
        "cb_delta": np.array([-0.0596, 0.0596], np.float32),
    }
    o = kernel(**fake)
    print("ran", o.shape, o.dtype)
